# revision 29
# baseline (speedup 1.0000x reference)
"""Trainium2 Bass kernel v4: 12-layer BERT-base forward + per-sample annotator
head. Data-parallel across 8 NeuronCores (8 sequences / 2048 tokens per core,
no collectives).

v4 changes vs v2 (trace-driven; 6.9ms -> 5.6ms on-chip):
- PE executes in order, so emission order IS the schedule. The scalar-bound
  attention phase (exp softmax) is filled with explicitly interleaved PE work:
  QKV GEMM groups of pair p+1 and Wo+LN1-stats groups of pair p-1 are emitted
  between attention head-pair units.
- All LN2 ln/exp are deferred past the last gelu of the layer (enforced by a
  real data dep: the Ln bias reads a gate tile computed from the last gelu
  output - emission order alone is not enough, the Tile scheduler is
  readiness-greedy); LN1 ln/exp all happen during the (nle-set) attention
  phase. 2 ACT table switches per layer instead of 8.
- W1/W2 weight-stream pools deepened to 4 bufs: with 2 the DMA chunk latency
  (~2.5us) was right at the consumption period, stalling the PE ~570us/run.
- Single f16 master per stream (xs_hi = LN2 out, xs1 = LN1 out) used directly
  as GEMM moving operand; pre-LN residuals are written INTO those tensors and
  LayerNorm is applied in place (x*a then +b), so there is no separate
  hpre/GEMM-input storage at all.
- QK PSUM evacuations on the scalar engine (Copy), V/ctx/residual on DVE:
  balances the two evacuation engines against the PE GEMM rate.
- Per-pair tiles (q/k/v, ctx) double-buffered so pair p+1 compute overlaps
  pair p consumption. Result: 97%+ PE busy, <150us total PE idle; remaining
  runtime variance is the board-level power throttle (gpio_1, 13/16 clock).
"""
import os
import ml_dtypes
import numpy as np

import concourse.bass as bass
import concourse.mybir as mybir
from concourse.tile import TileContext
from concourse.bass_utils import run_bass_kernel_spmd

# model dims (hardcoded per problem spec)
B, S, H, NLAYER, NH, VOC, ANN, NL = 64, 256, 768, 12, 12, 30522, 64, 2
HD = H // NH            # 64
FF = 4 * H              # 3072
P = 128
CH = H // P             # 6
FCH = FF // P           # 24
HF = FCH // 2           # 12
NCORES = 8
NB = B // NCORES        # 8 sequences per core
T = NB * S              # 2048 tokens per core
TB = 512                # token block == pair of sequences
NTB = T // TB           # 4
PAIR = 2 * S            # 512
EPS = 1e-12

F32 = mybir.dt.float32
F16 = mybir.dt.float16
BF16 = mybir.dt.bfloat16
AF = mybir.ActivationFunctionType
ALU = mybir.AluOpType

_NLAYERS = int(os.environ.get("KERNEL_LAYERS", str(NLAYER)))


# ---------------------------------------------------------------- wait split
def _split_sync_waits(nc, max_waits=1):
    """This walrus build rejects >~2 sync waits on one instruction; move
    overflow waits onto wait-only NoOps inserted before, same engine."""
    ctr = 0
    for f in nc.m.functions:
        for bb in f.blocks:
            new_list, changed = [], False
            for inst in bb.instructions:
                si = inst.sync_info
                waits = list(si.on_wait) if si and si.on_wait else []
                if len(waits) > max_waits:
                    changed = True
                    overflow = waits[: len(waits) - max_waits]
                    keep = waits[len(waits) - max_waits:]
                    for i in range(0, len(overflow), max_waits):
                        ctr += 1
                        nop = mybir.InstNoOp(name=f"waitsplit-{ctr}")
                        nop.engine = inst.engine
                        nop.sync_info = mybir.SyncInfo(
                            on_wait=overflow[i:i + max_waits], on_update=[])
                        nc.register_instruction(nop)
                        new_list.append(nop)
                    si.on_wait = keep
                    inst.sync_info = si
                new_list.append(inst)
            if changed:
                bb.instructions = new_list


# ---------------------------------------------------------------- host prep
def _tile_kxo(w, k, o):
    # [k, o] -> [128, k/128, o/128, 128] (kp, ko, oo, oc)
    return np.ascontiguousarray(
        w.reshape(k // P, P, o // P, P).transpose(1, 0, 2, 3))


def _rows_k(w, k, o):
    # [k, o] -> [128, k/128, o] (kp, ko, o)
    return np.ascontiguousarray(w.reshape(k // P, P, o).transpose(1, 0, 2))


def _w2_grouped(w):
    # [FF, H] -> [H/128 (oo), 128 (kp), FF/128 (ko), 128 (oc)]
    return np.ascontiguousarray(
        w.reshape(FCH, P, CH, P).transpose(2, 1, 0, 3))


def _f16(x):
    return np.asarray(x, np.float32).astype(ml_dtypes.bfloat16)


# ---------------------------------------------------------------- builder
def build(nl: int):
    nc = bass.Bass(target_bir_lowering=False)

    h0_d = nc.declare_dram_parameter("h0", [P, CH, T], BF16, isOutput=False)
    wq_d = nc.declare_dram_parameter("wq", [nl, P, CH, CH, P], BF16, isOutput=False)
    wk_d = nc.declare_dram_parameter("wk", [nl, P, CH, CH, P], BF16, isOutput=False)
    wv_d = nc.declare_dram_parameter("wv", [nl, P, CH, H], BF16, isOutput=False)
    wo_d = nc.declare_dram_parameter("wo", [nl, P, CH, CH, P], BF16, isOutput=False)
    w1_d = nc.declare_dram_parameter("w1", [nl, P, CH, FCH, P], BF16, isOutput=False)
    w2_d = nc.declare_dram_parameter("w2", [nl, CH, P, FCH, P], BF16, isOutput=False)
    hw_d = nc.declare_dram_parameter("hw", [P, CH, 2 * NB], BF16, isOutput=False)
    out_d = nc.declare_dram_parameter("out", [NB, 2 * NB], F32, isOutput=True)

    from contextlib import ExitStack
    with TileContext(nc) as tc:
        with ExitStack() as ctx:
            persist = ctx.enter_context(tc.tile_pool(name="persist", bufs=1))
            qkv_pool = ctx.enter_context(tc.tile_pool(name="qkvp", bufs=2))
            ctx_pool = ctx.enter_context(tc.tile_pool(name="ctxp", bufs=2))
            w768_pool = ctx.enter_context(tc.tile_pool(name="w768", bufs=1))
            w1_pool = ctx.enter_context(tc.tile_pool(name="w1s", bufs=4))
            w2_pool = ctx.enter_context(tc.tile_pool(name="w2s", bufs=4))
            g_pool = ctx.enter_context(tc.tile_pool(name="gp", bufs=1))
            scr_pool = ctx.enter_context(tc.tile_pool(name="scr", bufs=2))
            ln_pool = ctx.enter_context(tc.tile_pool(name="lnsm", bufs=2))
            ln2_pool = ctx.enter_context(tc.tile_pool(name="ln2sm", bufs=4))
            at_pool = ctx.enter_context(tc.tile_pool(name="attn", bufs=3))
            rec_pool = ctx.enter_context(tc.tile_pool(name="recp", bufs=2))
            ps_mm = ctx.enter_context(tc.tile_pool(name="psmm", bufs=3, space="PSUM"))
            ps_attn = ctx.enter_context(tc.tile_pool(name="psattn", bufs=5, space="PSUM"))

            ones16 = persist.tile([P, P], BF16)
            nc.vector.memset(ones16[:], 1.0)
            eps_t = persist.tile([P, 1], F32)
            nc.vector.memset(eps_t[:], EPS)
            xs_hi = persist.tile([P, CH, T], F16)   # master: LN2 out / QKV in
            xs1 = persist.tile([P, CH, T], F16)     # LN1 out: W1 in + residual

            # ---------- LN building blocks ----------
            def stats_pass(src):
                """6 ones-matmuls summing src [P,CH,TB] over partitions."""
                ps = ps_mm.tile([P, TB], F32, tag="pm")
                for c in range(CH):
                    nc.tensor.matmul(ps[:], ones16[:], src[:, c],
                                     start=(c == 0), stop=(c == CH - 1))
                return ps

            def sumsq_pass(src):
                """Per-chunk square (1K scratch) + ones-matmul accumulate."""
                ps = ps_mm.tile([P, TB], F32, tag="pm")
                for c in range(CH):
                    sq = scr_pool.tile([P, TB], BF16, tag="sq1")
                    nc.vector.tensor_tensor(sq[:], src[:, c], src[:, c],
                                            ALU.mult)
                    nc.tensor.matmul(ps[:], ones16[:], sq[:],
                                     start=(c == 0), stop=(c == CH - 1))
                return ps

            def ln_smalls(ps_s, ps_ss, pool):
                """mneg = -mean (f16), msq = var (f32). Copy/Square/stt only -
                present in every ACT table set."""
                mneg = pool.tile([P, TB], F16, tag="mneg")
                nc.scalar.activation(mneg[:], ps_s[:], AF.Copy, scale=-1.0 / H)
                msq = pool.tile([P, TB], F16, tag="msq")
                nc.scalar.activation(msq[:], mneg[:], AF.Square)
                nc.vector.scalar_tensor_tensor(msq[:], ps_ss[:], 1.0 / H,
                                               msq[:], ALU.mult, ALU.subtract)
                return mneg, msq

            def ln_rstd(msq, gate=None):
                """a = rsqrt(var+eps) via Ln/Exp (natural_log_exp set).
                `gate` (an eps-valued [P,1] tile) adds a data dependency so
                the readiness-greedy scheduler cannot hoist this Ln between
                gelus (ACT table thrash)."""
                b = gate if gate is not None else eps_t
                nc.scalar.activation(msq[:], msq[:], AF.Ln, bias=b[:, 0:1])
                a_t = ln_pool.tile([P, TB], F16, tag="a16")
                nc.scalar.activation(a_t[:], msq[:], AF.Exp, scale=-0.5)
                return a_t

            def ln_apply(src, mneg, a_t, dst):
                """dst[:, c] = (src[:, c] + mneg) * a_t = src*a + (mneg*a),
                f16 out, no full-size temp."""
                b_t = ln_pool.tile([P, TB], F16, tag="bt")
                nc.vector.tensor_tensor(b_t[:], mneg[:], a_t[:], ALU.mult)
                for c in range(CH):
                    nc.vector.tensor_tensor(dst[:, c], src[:, c], a_t[:],
                                            ALU.mult)
                for c in range(CH):
                    nc.vector.tensor_tensor(dst[:, c], dst[:, c], b_t[:],
                                            ALU.add)

            # ---------- embedding LN -> xs_hi ----------
            # h0 blocks staged in the (idle) double-buffered qkv pool
            for tb in range(NTB):
                sl = slice(tb * TB, (tb + 1) * TB)
                c16 = qkv_pool.tile([P, CH, PAIR], BF16, tag="qtb")
                nc.sync.dma_start(c16[:], h0_d[:, :, sl])
                ps_s = stats_pass(c16)
                ps_ss = sumsq_pass(c16)
                mneg, msq = ln_smalls(ps_s, ps_ss, ln_pool)
                a_t = ln_rstd(msq)
                ln_apply(c16, mneg, a_t, xs_hi[:, :, sl])

            # ---------------- layers ----------------
            for l in range(nl):
                wq_t = w768_pool.tile([P, CH, CH, P], BF16, tag="wq")
                nc.sync.dma_start(wq_t[:], wq_d[l])
                wk_t = w768_pool.tile([P, CH, CH, P], BF16, tag="wk")
                nc.sync.dma_start(wk_t[:], wk_d[l])
                wv_t = w768_pool.tile([P, CH, H], BF16, tag="wv")
                nc.sync.dma_start(wv_t[:], wv_d[l])
                wo_t = w768_pool.tile([P, CH, CH, P], BF16, tag="wo")
                nc.sync.dma_start(wo_t[:], wo_d[l])

                # per-pair state created lazily, keyed by pair index
                qkv_tiles = {}
                ctx_tiles = {}

                def qkv_group_fillers(pr, wq_t=wq_t, wk_t=wk_t, wv_t=wv_t,
                                      qkv_tiles=qkv_tiles):
                    """20 closures: QK 12 groups (scalar evac), V 8 (DVE)."""
                    psl = slice(pr * PAIR, (pr + 1) * PAIR)
                    qt_b = qkv_pool.tile([P, CH, PAIR], BF16, tag="qtb")
                    kt_b = qkv_pool.tile([P, CH, PAIR], BF16, tag="ktb")
                    v_b = qkv_pool.tile([P, 2, 2, NH, HD], BF16, tag="vb")
                    qkv_tiles[pr] = (qt_b, kt_b, v_b)
                    fillers = []

                    def qk_group(w_t, dst, o):
                        def go():
                            ps = ps_mm.tile([P, TB], F32, tag="pm")
                            for k in range(CH):
                                nc.tensor.matmul(ps[:], w_t[:, k, o],
                                                 xs_hi[:, k, psl],
                                                 start=(k == 0),
                                                 stop=(k == CH - 1))
                            nc.scalar.activation(dst[:, o], ps[:], AF.Copy)
                        return go

                    def v_group(ci, dh):
                        def go():
                            csl = slice(pr * PAIR + ci * P,
                                        pr * PAIR + (ci + 1) * P)
                            bi, kt_i = ci // 2, ci % 2
                            ps = ps_mm.tile([P, TB], F32, tag="pm")
                            for k in range(CH):
                                nc.tensor.matmul(
                                    ps[:, : H // 2],
                                    xs_hi[:, k, csl],
                                    wv_t[:, k, dh * (H // 2):(dh + 1) * (H // 2)],
                                    start=(k == 0), stop=(k == CH - 1))
                            nc.vector.tensor_copy(
                                v_b[:, bi, kt_i, dh * 6:(dh + 1) * 6],
                                ps[:, : H // 2].rearrange(
                                    "p (h d) -> p h d", d=HD))
                        return go

                    for o in range(CH):
                        fillers.append(qk_group(wq_t, qt_b, o))
                        fillers.append(qk_group(wk_t, kt_b, o))
                    for ci in range(PAIR // P):
                        for dh in range(2):
                            fillers.append(v_group(ci, dh))
                    return fillers

                def wo_ln1_fillers(pr, wo_t=wo_t, ctx_tiles=ctx_tiles):
                    """8 closures: 6 Wo groups (DVE residual evac into xs1,
                    pre-LN) then sum-stats / sumsq-stats; last closure
                    finishes LN1 (rstd nle + in-place DVE apply on xs1)."""
                    sl = slice(pr * TB, (pr + 1) * TB)
                    hp = xs1[:, :, sl]
                    fillers = []

                    def wo_group(o):
                        def go():
                            ps = ps_mm.tile([P, TB], F32, tag="pm")
                            ctx_t = ctx_tiles[pr]
                            for k in range(CH):
                                nc.tensor.matmul(ps[:], wo_t[:, k, o],
                                                 ctx_t[:, k],
                                                 start=(k == 0),
                                                 stop=(k == CH - 1))
                            nc.vector.scalar_tensor_tensor(
                                hp[:, o], ps[:], 1.0, xs_hi[:, o, sl],
                                ALU.mult, ALU.add)
                        return go

                    state = {}

                    def sum_stats():
                        state["ps_s"] = stats_pass(hp)

                    def sumsq_stats():
                        ps_ss = sumsq_pass(hp)
                        mneg, msq = ln_smalls(state["ps_s"], ps_ss, ln_pool)
                        a_t = ln_rstd(msq)
                        ln_apply(hp, mneg, a_t, hp)

                    for o in range(CH):
                        fillers.append(wo_group(o))
                    fillers.append(sum_stats)
                    fillers.append(sumsq_stats)
                    return fillers

                # ---------- attention machinery (as v2) ----------
                def attn_pair(pr, fillers, qkv_tiles=qkv_tiles,
                              ctx_tiles=ctx_tiles):
                    qt_b, kt_b, v_b = qkv_tiles.pop(pr)
                    ctx_t = ctx_pool.tile([P, CH, PAIR], BF16, tag="ctx")
                    ctx_tiles[pr] = ctx_t
                    hps = [(bi, j) for bi in range(2) for j in range(NH // 2)]

                    def issue_scores(hp):
                        bi, j = hp
                        qsl = slice(bi * S, (bi + 1) * S)
                        ats = []
                        scs = []
                        for hx in range(2):          # po = 0 / 64
                            sc_t = ps_attn.tile([P, 2, S], F32, tag="pa")
                            scs.append(sc_t)
                        for kt_i in range(2):
                            ksl = slice(bi * S + kt_i * P,
                                        bi * S + (kt_i + 1) * P)
                            for hx in range(2):
                                po = hx * HD
                                nc.tensor.matmul(
                                    scs[hx][:, kt_i],
                                    kt_b[po:po + HD, j, ksl],
                                    qt_b[po:po + HD, j, qsl],
                                    start=True, stop=True,
                                    tile_position=(po, 0))
                        for hx in range(2):
                            at = at_pool.tile([P, 2, S], BF16, tag="at")
                            nc.scalar.activation(at[:], scs[hx][:], AF.Exp,
                                                 scale=1.0 / np.sqrt(HD))
                            ats.append(at)
                        return ats

                    def issue_rest(hp, ats):
                        bi, j = hp
                        ps_sum = ps_attn.tile([P, S], F32, tag="pa")
                        for kt_i in range(2):
                            for hx in range(2):
                                po = hx * HD
                                nc.tensor.matmul(ps_sum[po:po + HD],
                                                 ones16[:, 0:HD],
                                                 ats[hx][:, kt_i],
                                                 start=(kt_i == 0),
                                                 stop=(kt_i == 1),
                                                 tile_position=(0, po))
                        lns = rec_pool.tile([P, S], F32, tag="lns")
                        nc.scalar.activation(lns[:], ps_sum[:], AF.Ln)
                        rec = rec_pool.tile([P, S], F32, tag="rec")
                        nc.scalar.activation(rec[:], lns[:], AF.Exp,
                                             scale=-1.0)
                        ps_ctx = ps_attn.tile([P, S], F32, tag="pa")
                        for kt_i in range(2):
                            for hx in range(2):
                                po = hx * HD
                                nc.tensor.matmul(
                                    ps_ctx[po:po + HD],
                                    v_b[:, bi, kt_i, 2 * j + hx],
                                    ats[hx][:, kt_i],
                                    start=(kt_i == 0), stop=(kt_i == 1),
                                    tile_position=(0, po))
                        nc.vector.tensor_tensor(ctx_t[:, j, bi * S:(bi + 1) * S],
                                                ps_ctx[:], rec[:], ALU.mult)

                    pend = {}
                    nf = len(fillers)
                    taken = 0
                    for i in range(len(hps) + 1):
                        if i < len(hps):
                            pend[i] = issue_scores(hps[i])
                        want = nf * (i + 1) // (len(hps) + 1)
                        while taken < want:
                            fillers[taken]()
                            taken += 1
                        if i >= 1:
                            issue_rest(hps[i - 1], pend.pop(i - 1))
                    while taken < nf:
                        fillers[taken]()
                        taken += 1

                # ---------- phase A: pairs with interleaved fillers ----------
                for f in qkv_group_fillers(0):
                    f()
                for pr in range(NTB):
                    fillers = []
                    if pr < NTB - 1:
                        fillers += qkv_group_fillers(pr + 1)
                    if pr >= 1:
                        fillers += wo_ln1_fillers(pr - 1)
                    attn_pair(pr, fillers)
                for f in wo_ln1_fillers(NTB - 1):
                    f()

                # ---------- phase B: FFN blocks ----------
                ln2_state = {}
                last_g = None
                for tb in range(NTB):
                    sl = slice(tb * TB, (tb + 1) * TB)
                    g = g_pool.tile([P, FCH, TB], BF16, tag="g")
                    for fog in range(HF):
                        w1_t = w1_pool.tile([P, CH, 2, P], BF16, tag="w1")
                        nc.sync.dma_start(
                            w1_t[:], w1_d[l, :, :, fog * 2:(fog + 1) * 2, :])
                        for fi in range(2):
                            fo = fog * 2 + fi
                            ps = ps_mm.tile([P, TB], F32, tag="pm")
                            for k in range(CH):
                                nc.tensor.matmul(ps[:], w1_t[:, k, fi],
                                                 xs1[:, k, sl],
                                                 start=(k == 0),
                                                 stop=(k == CH - 1))
                            nc.scalar.activation(g[:, fo], ps[:], AF.Gelu)
                            last_g = g
                    # W2 + residual -> pre-LN2 written into xs_hi (dead here)
                    hp2 = xs_hi[:, :, sl]
                    for o in range(CH):
                        ps = ps_mm.tile([P, TB], F32, tag="pm")
                        for kh in range(2):
                            w2_t = w2_pool.tile([P, HF, P], BF16, tag="w2")
                            nc.sync.dma_start(
                                w2_t[:], w2_d[l, o, :, kh * HF:(kh + 1) * HF])
                            for ki in range(HF):
                                k = kh * HF + ki
                                nc.tensor.matmul(ps[:], w2_t[:, ki], g[:, k],
                                                 start=(k == 0),
                                                 stop=(k == FCH - 1))
                        nc.vector.scalar_tensor_tensor(
                            hp2[:, o], ps[:], 1.0, xs1[:, o, sl],
                            ALU.mult, ALU.add)
                    ps_s = stats_pass(hp2)
                    ps_ss = sumsq_pass(hp2)
                    mneg, msq = ln_smalls(ps_s, ps_ss, ln2_pool)
                    ln2_state[tb] = (mneg, msq)

                # ---------- deferred LN2 (after last gelu): nle set ----------
                # gate = (g_last*0)+eps: real data dep on the layer's last gelu
                gate = ln_pool.tile([P, 1], F32, tag="gate")
                nc.vector.scalar_tensor_tensor(
                    gate[:], last_g[:, FCH - 1, 0:1], 0.0, eps_t[:],
                    ALU.mult, ALU.add)
                for tb in range(NTB):
                    sl = slice(tb * TB, (tb + 1) * TB)
                    mneg, msq = ln2_state[tb]
                    a_t = ln_rstd(msq, gate=gate)
                    ln_apply(xs_hi[:, :, sl], mneg, a_t, xs_hi[:, :, sl])

            # ---- head ----
            hw_sb = persist.tile([P, CH, 2 * NB], BF16)
            nc.sync.dma_start(hw_sb[:], hw_d[:])
            cls = persist.tile([P, CH, NB], BF16)
            for c in range(CH):
                nc.vector.tensor_copy(cls[:, c], xs_hi[:, c, 0:T:S])
            ps = ps_attn.tile([P, 2 * NB], F32, tag="pa")
            for c in range(CH):
                nc.tensor.matmul(ps[0:NB], cls[:, c], hw_sb[:, c],
                                 start=(c == 0), stop=(c == CH - 1))
            res = persist.tile([NB, 2 * NB], F32)
            nc.scalar.activation(res[:], ps[0:NB], AF.Copy)
            nc.sync.dma_start(out_d[:], res[:])

    _split_sync_waits(nc, max_waits=1)
    return nc


def _prep_weights(inputs, nl):
    wq = np.stack([_tile_kxo(_f16(inputs["Wq"][i]), H, H) for i in range(nl)])
    wk = np.stack([_tile_kxo(_f16(inputs["Wk"][i]), H, H) for i in range(nl)])
    wv = np.stack([_rows_k(_f16(inputs["Wv"][i]), H, H) for i in range(nl)])
    wo = np.stack([_tile_kxo(_f16(inputs["Wo"][i]), H, H) for i in range(nl)])
    w1 = np.stack([_tile_kxo(_f16(inputs["W1"][i]), H, FF) for i in range(nl)])
    w2 = np.stack([_w2_grouped(_f16(inputs["W2"][i])) for i in range(nl)])
    return wq, wk, wv, wo, w1, w2


def kernel(**inputs):
    nl = _NLAYERS
    for name in ("bq", "bk", "bv", "bo", "b1", "b2", "emb_ln_b", "head_b",
                 "ln1_b", "ln2_b"):
        assert not np.any(np.asarray(inputs[name])), f"{name} nonzero: unsupported"
    for name in ("emb_ln_s", "ln1_s", "ln2_s"):
        assert np.all(np.asarray(inputs[name]) == 1.0), f"{name}!=1: unsupported"
    assert np.all(np.asarray(inputs["attention_mask"]) == 1), "mask unsupported"

    ids = np.asarray(inputs["input_ids"])
    tt = np.asarray(inputs["token_type_ids"])
    we = np.asarray(inputs["word_emb"], np.float32)
    pe = np.asarray(inputs["pos_emb"], np.float32)
    te = np.asarray(inputs["type_emb"], np.float32)
    annot = np.asarray(inputs["annotator_idx"])
    hW = np.asarray(inputs["head_W"], np.float32)

    emb = we[ids] + pe[:S][None] + te[tt]          # [B, S, H] f32
    wq, wk, wv, wo, w1, w2 = _prep_weights(inputs, nl)

    in_maps = []
    for c in range(NCORES):
        e = emb[c * NB:(c + 1) * NB].reshape(T, CH, P).transpose(2, 1, 0)
        hw_g = _f16(hW[annot[c * NB:(c + 1) * NB]])  # [NB, H, 2]
        hwt = hw_g.transpose(1, 0, 2).reshape(H, 2 * NB) \
            .reshape(CH, P, 2 * NB).transpose(1, 0, 2)
        in_maps.append({
            "h0": np.ascontiguousarray(e).astype(ml_dtypes.bfloat16),
            "wq": wq, "wk": wk, "wv": wv, "wo": wo, "w1": w1, "w2": w2,
            "hw": np.ascontiguousarray(hwt),
        })

    nc = build(nl)

    trace = bool(int(os.environ.get("KERNEL_TRACE", "0")))
    kwargs = {}
    if trace:
        try:
            import profshim
            profshim.install()
            kwargs["tmpdir"] = os.environ.get("KERNEL_TRACE_DIR")
        except Exception:
            trace = False
    res = run_bass_kernel_spmd(nc, in_maps, core_ids=list(range(NCORES)),
                               trace=trace, **kwargs)
    kernel.last_exec_time_ns = res.exec_time_ns

    out = np.zeros((B, NL), np.float32)
    for c in range(NCORES):
        oc = res.results[c]["out"]                 # [NB, 2*NB]
        for b in range(NB):
            out[c * NB + b] = oc[b, 2 * b:2 * b + 2]
    return out


# revision 31
# speedup vs baseline: 1.1795x; 1.1795x over previous
"""Trainium2 Bass kernel v4: 12-layer BERT-base forward + per-sample annotator
head. Data-parallel across 8 NeuronCores (8 sequences / 2048 tokens per core,
no collectives).

v4 changes vs v2 (trace-driven; 6.9ms -> 5.6ms on-chip):
- PE executes in order, so emission order IS the schedule. The scalar-bound
  attention phase (exp softmax) is filled with explicitly interleaved PE work:
  QKV GEMM groups of pair p+1 and Wo+LN1-stats groups of pair p-1 are emitted
  between attention head-pair units.
- All LN2 ln/exp are deferred past the last gelu of the layer (enforced by a
  real data dep: the Ln bias reads a gate tile computed from the last gelu
  output - emission order alone is not enough, the Tile scheduler is
  readiness-greedy); LN1 ln/exp all happen during the (nle-set) attention
  phase. 2 ACT table switches per layer instead of 8.
- W1/W2 weight-stream pools deepened to 4 bufs: with 2 the DMA chunk latency
  (~2.5us) was right at the consumption period, stalling the PE ~570us/run.
- Single f16 master per stream (xs_hi = LN2 out, xs1 = LN1 out) used directly
  as GEMM moving operand; pre-LN residuals are written INTO those tensors and
  LayerNorm is applied in place (x*a then +b), so there is no separate
  hpre/GEMM-input storage at all.
- QK PSUM evacuations on the scalar engine (Copy), V/ctx/residual on DVE:
  balances the two evacuation engines against the PE GEMM rate.
- Per-pair tiles (q/k/v, ctx) double-buffered so pair p+1 compute overlaps
  pair p consumption. Result: 97%+ PE busy, <150us total PE idle; remaining
  runtime variance is the board-level power throttle (gpio_1, 13/16 clock).
"""
import os
import ml_dtypes
import numpy as np

import concourse.bass as bass
import concourse.mybir as mybir
from concourse.tile import TileContext
from concourse.bass_utils import run_bass_kernel_spmd

# model dims (hardcoded per problem spec)
B, S, H, NLAYER, NH, VOC, ANN, NL = 64, 256, 768, 12, 12, 30522, 64, 2
HD = H // NH            # 64
FF = 4 * H              # 3072
P = 128
CH = H // P             # 6
FCH = FF // P           # 24
HF = FCH // 2           # 12
NCORES = 8
NB = B // NCORES        # 8 sequences per core
T = NB * S              # 2048 tokens per core
TB = 512                # token block == pair of sequences
NTB = T // TB           # 4
PAIR = 2 * S            # 512
EPS = 1e-12

F32 = mybir.dt.float32
F16 = mybir.dt.float16
BF16 = mybir.dt.bfloat16
AF = mybir.ActivationFunctionType
ALU = mybir.AluOpType

_NLAYERS = int(os.environ.get("KERNEL_LAYERS", str(NLAYER)))


# ---------------------------------------------------------------- wait split
def _split_sync_waits(nc, max_waits=1):
    """This walrus build rejects >~2 sync waits on one instruction; move
    overflow waits onto wait-only NoOps inserted before, same engine."""
    ctr = 0
    for f in nc.m.functions:
        for bb in f.blocks:
            new_list, changed = [], False
            for inst in bb.instructions:
                si = inst.sync_info
                waits = list(si.on_wait) if si and si.on_wait else []
                if len(waits) > max_waits:
                    changed = True
                    overflow = waits[: len(waits) - max_waits]
                    keep = waits[len(waits) - max_waits:]
                    for i in range(0, len(overflow), max_waits):
                        ctr += 1
                        nop = mybir.InstNoOp(name=f"waitsplit-{ctr}")
                        nop.engine = inst.engine
                        nop.sync_info = mybir.SyncInfo(
                            on_wait=overflow[i:i + max_waits], on_update=[])
                        nc.register_instruction(nop)
                        new_list.append(nop)
                    si.on_wait = keep
                    inst.sync_info = si
                new_list.append(inst)
            if changed:
                bb.instructions = new_list


# ---------------------------------------------------------------- host prep
def _tile_kxo(w, k, o):
    # [k, o] -> [128, k/128, o/128, 128] (kp, ko, oo, oc)
    return np.ascontiguousarray(
        w.reshape(k // P, P, o // P, P).transpose(1, 0, 2, 3))


def _rows_k(w, k, o):
    # [k, o] -> [128, k/128, o] (kp, ko, o)
    return np.ascontiguousarray(w.reshape(k // P, P, o).transpose(1, 0, 2))


def _w2_grouped(w):
    # [FF, H] -> [H/128 (oo), 128 (kp), FF/128 (ko), 128 (oc)]
    return np.ascontiguousarray(
        w.reshape(FCH, P, CH, P).transpose(2, 1, 0, 3))


def _f16(x):
    return np.asarray(x, np.float32).astype(ml_dtypes.bfloat16)


# ---------------------------------------------------------------- builder
def build(nl: int):
    nc = bass.Bass(target_bir_lowering=False)

    h0_d = nc.declare_dram_parameter("h0", [P, CH, T], BF16, isOutput=False)
    wq_d = nc.declare_dram_parameter("wq", [nl, P, CH, CH, P], BF16, isOutput=False)
    wk_d = nc.declare_dram_parameter("wk", [nl, P, CH, CH, P], BF16, isOutput=False)
    wv_d = nc.declare_dram_parameter("wv", [nl, P, CH, H], BF16, isOutput=False)
    wo_d = nc.declare_dram_parameter("wo", [nl, P, CH, CH, P], BF16, isOutput=False)
    w1_d = nc.declare_dram_parameter("w1", [nl, P, CH, FCH, P], BF16, isOutput=False)
    w2_d = nc.declare_dram_parameter("w2", [nl, CH, P, FCH, P], BF16, isOutput=False)
    hw_d = nc.declare_dram_parameter("hw", [P, CH, 2 * NB], BF16, isOutput=False)
    out_d = nc.declare_dram_parameter("out", [NB, 2 * NB], F32, isOutput=True)

    from contextlib import ExitStack
    with TileContext(nc) as tc:
        with ExitStack() as ctx:
            persist = ctx.enter_context(tc.tile_pool(name="persist", bufs=1))
            qkv_pool = ctx.enter_context(tc.tile_pool(name="qkvp", bufs=2))
            ctx_pool = ctx.enter_context(tc.tile_pool(name="ctxp", bufs=2))
            w768_pool = ctx.enter_context(tc.tile_pool(name="w768", bufs=1))
            w1_pool = ctx.enter_context(tc.tile_pool(name="w1s", bufs=4))
            w2_pool = ctx.enter_context(tc.tile_pool(name="w2s", bufs=4))
            g_pool = ctx.enter_context(tc.tile_pool(name="gp", bufs=1))
            scr_pool = ctx.enter_context(tc.tile_pool(name="scr", bufs=2))
            ln_pool = ctx.enter_context(tc.tile_pool(name="lnsm", bufs=2))
            ln2_pool = ctx.enter_context(tc.tile_pool(name="ln2sm", bufs=4))
            at_pool = ctx.enter_context(tc.tile_pool(name="attn", bufs=3))
            rec_pool = ctx.enter_context(tc.tile_pool(name="recp", bufs=2))
            ps_mm = ctx.enter_context(tc.tile_pool(name="psmm", bufs=3, space="PSUM"))
            ps_attn = ctx.enter_context(tc.tile_pool(name="psattn", bufs=5, space="PSUM"))

            ones16 = persist.tile([P, P], BF16)
            nc.vector.memset(ones16[:], 1.0)
            eps_t = persist.tile([P, 1], F32)
            nc.vector.memset(eps_t[:], EPS)
            xs_hi = persist.tile([P, CH, T], F16)   # master: LN2 out / QKV in
            xs1 = persist.tile([P, CH, T], F16)     # LN1 out: W1 in + residual

            # ---------- LN building blocks ----------
            def stats_pass(src):
                """6 ones-matmuls summing src [P,CH,TB] over partitions."""
                ps = ps_mm.tile([P, TB], F32, tag="pm")
                for c in range(CH):
                    nc.tensor.matmul(ps[:], ones16[:], src[:, c],
                                     start=(c == 0), stop=(c == CH - 1))
                return ps

            def sumsq_pass(src):
                """Per-chunk square (1K scratch) + ones-matmul accumulate."""
                ps = ps_mm.tile([P, TB], F32, tag="pm")
                for c in range(CH):
                    sq = scr_pool.tile([P, TB], BF16, tag="sq1")
                    nc.vector.tensor_tensor(sq[:], src[:, c], src[:, c],
                                            ALU.mult)
                    nc.tensor.matmul(ps[:], ones16[:], sq[:],
                                     start=(c == 0), stop=(c == CH - 1))
                return ps

            def ln_smalls(ps_s, ps_ss, pool):
                """mneg = -mean (f16), msq = var (f32). Copy/Square/stt only -
                present in every ACT table set."""
                mneg = pool.tile([P, TB], F16, tag="mneg")
                nc.scalar.activation(mneg[:], ps_s[:], AF.Copy, scale=-1.0 / H)
                msq = pool.tile([P, TB], F16, tag="msq")
                nc.scalar.activation(msq[:], mneg[:], AF.Square)
                nc.vector.scalar_tensor_tensor(msq[:], ps_ss[:], 1.0 / H,
                                               msq[:], ALU.mult, ALU.subtract)
                return mneg, msq

            def ln_rstd(msq, gate=None):
                """a = rsqrt(var+eps) via Ln/Exp (natural_log_exp set).
                `gate` (an eps-valued [P,1] tile) adds a data dependency so
                the readiness-greedy scheduler cannot hoist this Ln between
                gelus (ACT table thrash)."""
                b = gate if gate is not None else eps_t
                nc.scalar.activation(msq[:], msq[:], AF.Ln, bias=b[:, 0:1])
                a_t = ln_pool.tile([P, TB], F16, tag="a16")
                nc.scalar.activation(a_t[:], msq[:], AF.Exp, scale=-0.5)
                return a_t

            def ln_apply(src, mneg, a_t, dst):
                """dst[:, c] = (src[:, c] + mneg) * a_t = src*a + (mneg*a),
                f16 out, no full-size temp."""
                b_t = ln_pool.tile([P, TB], F16, tag="bt")
                nc.vector.tensor_tensor(b_t[:], mneg[:], a_t[:], ALU.mult)
                for c in range(CH):
                    nc.vector.tensor_tensor(dst[:, c], src[:, c], a_t[:],
                                            ALU.mult)
                for c in range(CH):
                    nc.vector.tensor_tensor(dst[:, c], dst[:, c], b_t[:],
                                            ALU.add)

            # ---------- embedding LN -> xs_hi ----------
            # h0 blocks staged in the (idle) double-buffered qkv pool
            for tb in range(NTB):
                sl = slice(tb * TB, (tb + 1) * TB)
                c16 = qkv_pool.tile([P, CH, PAIR], BF16, tag="qtb")
                nc.sync.dma_start(c16[:], h0_d[:, :, sl])
                ps_s = stats_pass(c16)
                ps_ss = sumsq_pass(c16)
                mneg, msq = ln_smalls(ps_s, ps_ss, ln_pool)
                a_t = ln_rstd(msq)
                ln_apply(c16, mneg, a_t, xs_hi[:, :, sl])

            # ---------------- layers ----------------
            for l in range(nl):
                wq_t = w768_pool.tile([P, CH, CH, P], BF16, tag="wq")
                nc.sync.dma_start(wq_t[:], wq_d[l])
                wk_t = w768_pool.tile([P, CH, CH, P], BF16, tag="wk")
                nc.sync.dma_start(wk_t[:], wk_d[l])
                wv_t = w768_pool.tile([P, CH, H], BF16, tag="wv")
                nc.sync.dma_start(wv_t[:], wv_d[l])
                wo_t = w768_pool.tile([P, CH, CH, P], BF16, tag="wo")
                nc.sync.dma_start(wo_t[:], wo_d[l])

                # per-pair state created lazily, keyed by pair index
                qkv_tiles = {}
                ctx_tiles = {}

                def qkv_group_fillers(pr, wq_t=wq_t, wk_t=wk_t, wv_t=wv_t,
                                      qkv_tiles=qkv_tiles):
                    """20 closures: QK 12 groups (scalar evac), V 8 (DVE)."""
                    psl = slice(pr * PAIR, (pr + 1) * PAIR)
                    qt_b = qkv_pool.tile([P, CH, PAIR], BF16, tag="qtb")
                    kt_b = qkv_pool.tile([P, CH, PAIR], BF16, tag="ktb")
                    v_b = qkv_pool.tile([P, 2, 2, NH, HD], BF16, tag="vb")
                    qkv_tiles[pr] = (qt_b, kt_b, v_b)
                    fillers = []

                    def qk_group(w_t, dst, o):
                        def go():
                            ps = ps_mm.tile([P, TB], F32, tag="pm")
                            for k in range(CH):
                                nc.tensor.matmul(ps[:], w_t[:, k, o],
                                                 xs_hi[:, k, psl],
                                                 start=(k == 0),
                                                 stop=(k == CH - 1))
                            nc.scalar.activation(dst[:, o], ps[:], AF.Copy)
                        return go

                    def v_group(ci, dh):
                        def go():
                            csl = slice(pr * PAIR + ci * P,
                                        pr * PAIR + (ci + 1) * P)
                            bi, kt_i = ci // 2, ci % 2
                            ps = ps_mm.tile([P, TB], F32, tag="pm")
                            for k in range(CH):
                                nc.tensor.matmul(
                                    ps[:, : H // 2],
                                    xs_hi[:, k, csl],
                                    wv_t[:, k, dh * (H // 2):(dh + 1) * (H // 2)],
                                    start=(k == 0), stop=(k == CH - 1))
                            nc.vector.tensor_copy(
                                v_b[:, bi, kt_i, dh * 6:(dh + 1) * 6],
                                ps[:, : H // 2].rearrange(
                                    "p (h d) -> p h d", d=HD))
                        return go

                    for o in range(CH):
                        fillers.append(qk_group(wq_t, qt_b, o))
                        fillers.append(qk_group(wk_t, kt_b, o))
                    for ci in range(PAIR // P):
                        for dh in range(2):
                            fillers.append(v_group(ci, dh))
                    return fillers

                def wo_ln1_fillers(pr, wo_t=wo_t, ctx_tiles=ctx_tiles):
                    """8 closures: 6 Wo groups (DVE residual evac into xs1,
                    pre-LN) then sum-stats / sumsq-stats; last closure
                    finishes LN1 (rstd nle + in-place DVE apply on xs1)."""
                    sl = slice(pr * TB, (pr + 1) * TB)
                    hp = xs1[:, :, sl]
                    fillers = []

                    def wo_group(o):
                        def go():
                            ps = ps_mm.tile([P, TB], F32, tag="pm")
                            ctx_t = ctx_tiles[pr]
                            for k in range(CH):
                                nc.tensor.matmul(ps[:], wo_t[:, k, o],
                                                 ctx_t[:, k],
                                                 start=(k == 0),
                                                 stop=(k == CH - 1))
                            nc.vector.scalar_tensor_tensor(
                                hp[:, o], ps[:], 1.0, xs_hi[:, o, sl],
                                ALU.mult, ALU.add)
                        return go

                    state = {}

                    def sum_stats():
                        state["ps_s"] = stats_pass(hp)

                    def sumsq_stats():
                        ps_ss = sumsq_pass(hp)
                        mneg, msq = ln_smalls(state["ps_s"], ps_ss, ln_pool)
                        a_t = ln_rstd(msq)
                        ln_apply(hp, mneg, a_t, hp)

                    for o in range(CH):
                        fillers.append(wo_group(o))
                    fillers.append(sum_stats)
                    fillers.append(sumsq_stats)
                    return fillers

                # ---------- attention machinery (as v2) ----------
                def attn_pair(pr, fillers, qkv_tiles=qkv_tiles,
                              ctx_tiles=ctx_tiles):
                    qt_b, kt_b, v_b = qkv_tiles.pop(pr)
                    ctx_t = ctx_pool.tile([P, CH, PAIR], BF16, tag="ctx")
                    ctx_tiles[pr] = ctx_t
                    hps = [(bi, j) for bi in range(2) for j in range(NH // 2)]

                    def issue_scores(hp):
                        bi, j = hp
                        qsl = slice(bi * S, (bi + 1) * S)
                        ats = []
                        scs = []
                        for hx in range(2):          # po = 0 / 64
                            sc_t = ps_attn.tile([P, 2, S], F32, tag="pa")
                            scs.append(sc_t)
                        for kt_i in range(2):
                            ksl = slice(bi * S + kt_i * P,
                                        bi * S + (kt_i + 1) * P)
                            for hx in range(2):
                                po = hx * HD
                                nc.tensor.matmul(
                                    scs[hx][:, kt_i],
                                    kt_b[po:po + HD, j, ksl],
                                    qt_b[po:po + HD, j, qsl],
                                    start=True, stop=True,
                                    tile_position=(po, 0))
                        for hx in range(2):
                            at = at_pool.tile([P, 2, S], BF16, tag="at")
                            nc.scalar.activation(at[:], scs[hx][:], AF.Exp,
                                                 scale=1.0 / np.sqrt(HD))
                            ats.append(at)
                        return ats

                    def issue_rest(hp, ats):
                        bi, j = hp
                        ps_sum = ps_attn.tile([P, S], F32, tag="pa")
                        for kt_i in range(2):
                            for hx in range(2):
                                po = hx * HD
                                nc.tensor.matmul(ps_sum[po:po + HD],
                                                 ones16[:, 0:HD],
                                                 ats[hx][:, kt_i],
                                                 start=(kt_i == 0),
                                                 stop=(kt_i == 1),
                                                 tile_position=(0, po))
                        lns = rec_pool.tile([P, S], F32, tag="lns")
                        nc.scalar.activation(lns[:], ps_sum[:], AF.Ln)
                        rec = rec_pool.tile([P, S], F32, tag="rec")
                        nc.scalar.activation(rec[:], lns[:], AF.Exp,
                                             scale=-1.0)
                        ps_ctx = ps_attn.tile([P, S], F32, tag="pa")
                        for kt_i in range(2):
                            for hx in range(2):
                                po = hx * HD
                                nc.tensor.matmul(
                                    ps_ctx[po:po + HD],
                                    v_b[:, bi, kt_i, 2 * j + hx],
                                    ats[hx][:, kt_i],
                                    start=(kt_i == 0), stop=(kt_i == 1),
                                    tile_position=(0, po))
                        nc.vector.tensor_tensor(ctx_t[:, j, bi * S:(bi + 1) * S],
                                                ps_ctx[:], rec[:], ALU.mult)

                    pend = {}
                    nf = len(fillers)
                    taken = 0
                    for i in range(len(hps) + 1):
                        if i < len(hps):
                            pend[i] = issue_scores(hps[i])
                        want = nf * (i + 1) // (len(hps) + 1)
                        while taken < want:
                            fillers[taken]()
                            taken += 1
                        if i >= 1:
                            issue_rest(hps[i - 1], pend.pop(i - 1))
                    while taken < nf:
                        fillers[taken]()
                        taken += 1

                # ---------- phase A: pairs with interleaved fillers ----------
                for f in qkv_group_fillers(0):
                    f()
                for pr in range(NTB):
                    fillers = []
                    if pr < NTB - 1:
                        fillers += qkv_group_fillers(pr + 1)
                    if pr >= 1:
                        fillers += wo_ln1_fillers(pr - 1)
                    attn_pair(pr, fillers)
                for f in wo_ln1_fillers(NTB - 1):
                    f()

                # ---------- phase B: FFN blocks ----------
                if l == nl - 1:
                    # The head reads only the CLS token of each sequence, so
                    # the last layer's FFN + LN2 run on the NB=8 CLS columns
                    # (t = 0, S, 2S, ...) instead of all 2048 tokens
                    # (~245us of PE work skipped).
                    NC = NB
                    xcls = persist.tile([P, CH, NC], F16)
                    for c in range(CH):
                        nc.vector.tensor_copy(xcls[:, c], xs1[:, c, 0:T:S])
                    gc = g_pool.tile([P, FCH, NC], BF16, tag="gc")
                    for fog in range(HF):
                        w1_t = w1_pool.tile([P, CH, 2, P], BF16, tag="w1")
                        nc.sync.dma_start(
                            w1_t[:], w1_d[l, :, :, fog * 2:(fog + 1) * 2, :])
                        for fi in range(2):
                            fo = fog * 2 + fi
                            ps = ps_mm.tile([P, TB], F32, tag="pm")
                            for k in range(CH):
                                nc.tensor.matmul(ps[:, 0:NC], w1_t[:, k, fi],
                                                 xcls[:, k],
                                                 start=(k == 0),
                                                 stop=(k == CH - 1))
                            nc.scalar.activation(gc[:, fo], ps[:, 0:NC],
                                                 AF.Gelu)
                    hc = persist.tile([P, CH, NC], F16)
                    for o in range(CH):
                        ps = ps_mm.tile([P, TB], F32, tag="pm")
                        for kh in range(2):
                            w2_t = w2_pool.tile([P, HF, P], BF16, tag="w2")
                            nc.sync.dma_start(
                                w2_t[:], w2_d[l, o, :, kh * HF:(kh + 1) * HF])
                            for ki in range(HF):
                                k = kh * HF + ki
                                nc.tensor.matmul(ps[:, 0:NC], w2_t[:, ki],
                                                 gc[:, k],
                                                 start=(k == 0),
                                                 stop=(k == FCH - 1))
                        nc.vector.scalar_tensor_tensor(
                            hc[:, o], ps[:, 0:NC], 1.0, xcls[:, o],
                            ALU.mult, ALU.add)
                    # LN2 on the 8 CLS columns; deps (stats <- all of hc <-
                    # all gelus) already order the Ln after the gelu set
                    ps_s = ps_mm.tile([P, TB], F32, tag="pm")
                    for c in range(CH):
                        nc.tensor.matmul(ps_s[:, 0:NC], ones16[:], hc[:, c],
                                         start=(c == 0), stop=(c == CH - 1))
                    ps_ss = ps_mm.tile([P, TB], F32, tag="pm")
                    for c in range(CH):
                        sq = scr_pool.tile([P, TB], BF16, tag="sq1")
                        nc.vector.tensor_tensor(sq[:, 0:NC], hc[:, c],
                                                hc[:, c], ALU.mult)
                        nc.tensor.matmul(ps_ss[:, 0:NC], ones16[:],
                                         sq[:, 0:NC],
                                         start=(c == 0), stop=(c == CH - 1))
                    mneg = ln_pool.tile([P, TB], F16, tag="mneg")
                    nc.scalar.activation(mneg[:, 0:NC], ps_s[:, 0:NC],
                                         AF.Copy, scale=-1.0 / H)
                    msq = ln_pool.tile([P, TB], F16, tag="msq")
                    nc.scalar.activation(msq[:, 0:NC], mneg[:, 0:NC],
                                         AF.Square)
                    nc.vector.scalar_tensor_tensor(
                        msq[:, 0:NC], ps_ss[:, 0:NC], 1.0 / H, msq[:, 0:NC],
                        ALU.mult, ALU.subtract)
                    nc.scalar.activation(msq[:, 0:NC], msq[:, 0:NC], AF.Ln,
                                         bias=eps_t[:, 0:1])
                    a_t = ln_pool.tile([P, TB], F16, tag="a16")
                    nc.scalar.activation(a_t[:, 0:NC], msq[:, 0:NC], AF.Exp,
                                         scale=-0.5)
                    b_t = ln_pool.tile([P, TB], F16, tag="bt")
                    nc.vector.tensor_tensor(b_t[:, 0:NC], mneg[:, 0:NC],
                                            a_t[:, 0:NC], ALU.mult)
                    cls_sb = persist.tile([P, CH, NC], BF16)
                    for c in range(CH):
                        nc.vector.tensor_tensor(cls_sb[:, c], hc[:, c],
                                                a_t[:, 0:NC], ALU.mult)
                    for c in range(CH):
                        nc.vector.tensor_tensor(cls_sb[:, c], cls_sb[:, c],
                                                b_t[:, 0:NC], ALU.add)
                    continue

                ln2_state = {}
                last_g = None
                for tb in range(NTB):
                    sl = slice(tb * TB, (tb + 1) * TB)
                    g = g_pool.tile([P, FCH, TB], BF16, tag="g")
                    for fog in range(HF):
                        w1_t = w1_pool.tile([P, CH, 2, P], BF16, tag="w1")
                        nc.sync.dma_start(
                            w1_t[:], w1_d[l, :, :, fog * 2:(fog + 1) * 2, :])
                        for fi in range(2):
                            fo = fog * 2 + fi
                            ps = ps_mm.tile([P, TB], F32, tag="pm")
                            for k in range(CH):
                                nc.tensor.matmul(ps[:], w1_t[:, k, fi],
                                                 xs1[:, k, sl],
                                                 start=(k == 0),
                                                 stop=(k == CH - 1))
                            nc.scalar.activation(g[:, fo], ps[:], AF.Gelu)
                            last_g = g
                    # W2 + residual -> pre-LN2 written into xs_hi (dead here)
                    hp2 = xs_hi[:, :, sl]
                    for o in range(CH):
                        ps = ps_mm.tile([P, TB], F32, tag="pm")
                        for kh in range(2):
                            w2_t = w2_pool.tile([P, HF, P], BF16, tag="w2")
                            nc.sync.dma_start(
                                w2_t[:], w2_d[l, o, :, kh * HF:(kh + 1) * HF])
                            for ki in range(HF):
                                k = kh * HF + ki
                                nc.tensor.matmul(ps[:], w2_t[:, ki], g[:, k],
                                                 start=(k == 0),
                                                 stop=(k == FCH - 1))
                        nc.vector.scalar_tensor_tensor(
                            hp2[:, o], ps[:], 1.0, xs1[:, o, sl],
                            ALU.mult, ALU.add)
                    ps_s = stats_pass(hp2)
                    ps_ss = sumsq_pass(hp2)
                    mneg, msq = ln_smalls(ps_s, ps_ss, ln2_pool)
                    ln2_state[tb] = (mneg, msq)

                # ---------- deferred LN2 (after last gelu): nle set ----------
                # gate = (g_last*0)+eps: real data dep on the layer's last gelu
                gate = ln_pool.tile([P, 1], F32, tag="gate")
                nc.vector.scalar_tensor_tensor(
                    gate[:], last_g[:, FCH - 1, 0:1], 0.0, eps_t[:],
                    ALU.mult, ALU.add)
                for tb in range(NTB):
                    sl = slice(tb * TB, (tb + 1) * TB)
                    mneg, msq = ln2_state[tb]
                    a_t = ln_rstd(msq, gate=gate)
                    ln_apply(xs_hi[:, :, sl], mneg, a_t, xs_hi[:, :, sl])

            # ---- head ----
            hw_sb = persist.tile([P, CH, 2 * NB], BF16)
            nc.sync.dma_start(hw_sb[:], hw_d[:])
            ps = ps_attn.tile([P, 2 * NB], F32, tag="pa")
            for c in range(CH):
                nc.tensor.matmul(ps[0:NB], cls_sb[:, c], hw_sb[:, c],
                                 start=(c == 0), stop=(c == CH - 1))
            res = persist.tile([NB, 2 * NB], F32)
            nc.scalar.activation(res[:], ps[0:NB], AF.Copy)
            nc.sync.dma_start(out_d[:], res[:])

    _split_sync_waits(nc, max_waits=1)
    return nc


def _prep_weights(inputs, nl):
    wq = np.stack([_tile_kxo(_f16(inputs["Wq"][i]), H, H) for i in range(nl)])
    wk = np.stack([_tile_kxo(_f16(inputs["Wk"][i]), H, H) for i in range(nl)])
    wv = np.stack([_rows_k(_f16(inputs["Wv"][i]), H, H) for i in range(nl)])
    wo = np.stack([_tile_kxo(_f16(inputs["Wo"][i]), H, H) for i in range(nl)])
    w1 = np.stack([_tile_kxo(_f16(inputs["W1"][i]), H, FF) for i in range(nl)])
    w2 = np.stack([_w2_grouped(_f16(inputs["W2"][i])) for i in range(nl)])
    return wq, wk, wv, wo, w1, w2


def kernel(**inputs):
    nl = _NLAYERS
    for name in ("bq", "bk", "bv", "bo", "b1", "b2", "emb_ln_b", "head_b",
                 "ln1_b", "ln2_b"):
        assert not np.any(np.asarray(inputs[name])), f"{name} nonzero: unsupported"
    for name in ("emb_ln_s", "ln1_s", "ln2_s"):
        assert np.all(np.asarray(inputs[name]) == 1.0), f"{name}!=1: unsupported"
    assert np.all(np.asarray(inputs["attention_mask"]) == 1), "mask unsupported"

    ids = np.asarray(inputs["input_ids"])
    tt = np.asarray(inputs["token_type_ids"])
    we = np.asarray(inputs["word_emb"], np.float32)
    pe = np.asarray(inputs["pos_emb"], np.float32)
    te = np.asarray(inputs["type_emb"], np.float32)
    annot = np.asarray(inputs["annotator_idx"])
    hW = np.asarray(inputs["head_W"], np.float32)

    emb = we[ids] + pe[:S][None] + te[tt]          # [B, S, H] f32
    wq, wk, wv, wo, w1, w2 = _prep_weights(inputs, nl)

    in_maps = []
    for c in range(NCORES):
        e = emb[c * NB:(c + 1) * NB].reshape(T, CH, P).transpose(2, 1, 0)
        hw_g = _f16(hW[annot[c * NB:(c + 1) * NB]])  # [NB, H, 2]
        hwt = hw_g.transpose(1, 0, 2).reshape(H, 2 * NB) \
            .reshape(CH, P, 2 * NB).transpose(1, 0, 2)
        in_maps.append({
            "h0": np.ascontiguousarray(e).astype(ml_dtypes.bfloat16),
            "wq": wq, "wk": wk, "wv": wv, "wo": wo, "w1": w1, "w2": w2,
            "hw": np.ascontiguousarray(hwt),
        })

    nc = build(nl)

    trace = bool(int(os.environ.get("KERNEL_TRACE", "0")))
    kwargs = {}
    if trace:
        try:
            import profshim
            profshim.install()
            kwargs["tmpdir"] = os.environ.get("KERNEL_TRACE_DIR")
        except Exception:
            trace = False
    res = run_bass_kernel_spmd(nc, in_maps, core_ids=list(range(NCORES)),
                               trace=trace, **kwargs)
    kernel.last_exec_time_ns = res.exec_time_ns

    out = np.zeros((B, NL), np.float32)
    for c in range(NCORES):
        oc = res.results[c]["out"]                 # [NB, 2*NB]
        for b in range(NB):
            out[c * NB + b] = oc[b, 2 * b:2 * b + 2]
    return out
